# revision 27
# baseline (speedup 1.0000x reference)
"""DeltaOnlyModel Trainium2 kernel, v4 — hardware loops + small chunks.

Measured cost model of this backend (with the persistent jax compilation
cache active — see the jax.config.update calls below; env vars alone are
ignored because the axon site hook pre-imports jax): wall ~= 110ms fixed
(~30ms pjit retrace per call + ~8x13.5ms serial per-device result-fetch
RPCs) + ~34us per STATIC instruction + ~16us/KB of host->device input
bytes + executed DVE ops at ~(1us + 0.5ns/elem). Executed matmuls, DMAs,
and loop back-edges are ~free. Hence:
- the chunk machinery and the per-step gated solve run inside tc.For_i
  hardware loops (static code emitted once, executed 64x / 2048x)
- chunk length C=32 minimizes per-step DVE elements (correction window
  = C); chunk-boundary cross-talk goes through the exact token-table
  update T = Vtab - Ktab M^T, so the algorithm is exact for any C
- A = K^T K is precomputed per chunk on the PE and fetched row-contiguous
  ([e, t, s]) via a DRAM-roundtrip transpose; u is stored transposed
  [e, h, s] so the correction multiply is fully contiguous
- W=1 solve, 5 DVE ops per step; a zero-initialized u buffer makes
  masking unnecessary (slots s >= t contribute 0)
- tokens ship as uint8; transposes avoid any identity-matrix input
"""

import os
os.environ.setdefault("JAX_COMPILATION_CACHE_DIR", "/tmp/jax_comp_cache")
os.environ.setdefault("JAX_PERSISTENT_CACHE_MIN_ENTRY_SIZE_BYTES", "0")
os.environ.setdefault("JAX_PERSISTENT_CACHE_MIN_COMPILE_TIME_SECS", "0")
import jax
jax.config.update("jax_compilation_cache_dir", "/tmp/jax_comp_cache")
jax.config.update("jax_persistent_cache_min_entry_size_bytes", 0)
jax.config.update("jax_persistent_cache_min_compile_time_secs", 0)

import numpy as np

H = 64
VOC = 64
L = 2048
B = 256
NCORE = 8
BPC = B // NCORE          # 32 examples per core
C = 32                    # chunk length (solve correction window)
NCH = L // C
THR2 = 0.4 * 0.4
LN_EPS = 1e-5
NORM_EPS = 1e-12

# all f32 constants ship as one [1, N] blob (per-param transfer overhead
# through the tunnel is ~1.8ms; 14 params -> 1 saves ~25ms/call)
CONST_SPECS = [
    ("embed", (VOC, H)), ("w1", (H, 2 * H)), ("b1", (2 * H, 1)),
    ("w2", (2 * H, H)), ("b2", (H, 1)), ("ln_g", (1, H)), ("ln_b", (1, H)),
    ("wk", (H, H)), ("wv", (H, H)), ("wq", (H, H)), ("wo", (H, H)),
    ("bo", (H, 1)), ("iotaf", (128, 1)),
]
CONST_OFF = {}
_off = 0
for _n, _s in CONST_SPECS:
    CONST_OFF[_n] = _off
    _off += _s[0] * _s[1]
NBLOB = _off


def _build(nc, tc, ctx):
    from concourse import mybir
    f32 = mybir.dt.float32
    u8 = mybir.dt.uint8
    AL = mybir.AluOpType
    AF = mybir.ActivationFunctionType
    E = BPC

    def inp(name, shape, dt=f32):
        return nc.dram_tensor(name, shape, dt, kind="ExternalInput").ap()

    xf = inp("xf", [E, L], u8)
    # each core ships 1/8 of the const blob; an on-device AllGather
    # reassembles the full blob in DRAM (saves ~1MB of tunnel transfer)
    cpart = inp("cpart", [1, NBLOB // NCORE])
    cstg = nc.dram_tensor("cstg", [1, NBLOB // NCORE], f32).ap()
    nc.sync.dma_start(cstg, cpart)
    cblob = nc.dram_tensor("cblob_d", [1, NBLOB], f32).ap()
    nc.gpsimd.collective_compute(
        "AllGather", mybir.AluOpType.bypass,
        [list(range(NCORE))], [cstg], [cblob])

    def cview(name):
        off = CONST_OFF[name]
        a, b = dict(CONST_SPECS)[name]
        return cblob[:, off:off + a * b].rearrange(
            "o (a b) -> (o a) b", a=a, b=b)

    embed = cview("embed")
    w1 = cview("w1")
    b1 = cview("b1")
    w2 = cview("w2")
    b2 = cview("b2")
    ln_g = cview("ln_g")
    ln_b = cview("ln_b")
    wk = cview("wk")
    wv = cview("wv")
    wq = cview("wq")
    wo = cview("wo")
    bo = cview("bo")
    iotaf = cview("iotaf")
    out_d = nc.dram_tensor("out", [H, E], f32, kind="ExternalOutput").ap()

    # DRAM scratch for layout transposes
    aneg_d = nc.dram_tensor("aneg_d", [C, E, C], f32).ap()
    rk_d = nc.dram_tensor("rk_d", [C, E, 2 * H], f32).ap()
    th_d = nc.dram_tensor("th_d", [E, C], f32).ap()
    utt_d = nc.dram_tensor("utt_d", [E, H, C], f32).ap()
    tr_d = nc.dram_tensor("tr_d", [128, 128], f32).ap()   # phase-0 transposes

    cst = ctx.enter_context(tc.tile_pool(name="cst", bufs=1))
    per = ctx.enter_context(tc.tile_pool(name="per", bufs=1))
    pst = ctx.enter_context(tc.tile_pool(name="pst", bufs=1, space="PSUM"))

    # ============ phase 0: token tables ============
    embT = cst.tile([H, VOC], f32)
    nc.sync.dma_start(embT[:], embed.rearrange("a b -> b a"))
    w1s = cst.tile([H, 2 * H], f32)
    nc.sync.dma_start(w1s[:], w1)
    b1s = cst.tile([2 * H, 1], f32)
    nc.sync.dma_start(b1s[:], b1)
    w2s = cst.tile([2 * H, H], f32)
    nc.sync.dma_start(w2s[:], w2)
    b2s = cst.tile([H, 1], f32)
    nc.sync.dma_start(b2s[:], b2)
    gRow = cst.tile([VOC, H], f32)
    nc.sync.dma_start(gRow[:], ln_g.broadcast_to([VOC, H]))
    bRow = cst.tile([VOC, H], f32)
    nc.sync.dma_start(bRow[:], ln_b.broadcast_to([VOC, H]))
    wks = cst.tile([H, H], f32)
    nc.sync.dma_start(wks[:], wk)
    wvs = cst.tile([H, H], f32)
    nc.sync.dma_start(wvs[:], wv)
    wqs = cst.tile([H, H], f32)
    nc.sync.dma_start(wqs[:], wq)
    wos = cst.tile([H, H], f32)
    nc.sync.dma_start(wos[:], wo)
    bos = cst.tile([H, 1], f32)
    nc.sync.dma_start(bos[:], bo)
    iotf = cst.tile([128, 1], f32)
    nc.sync.dma_start(iotf[:], iotaf)

    ps1 = pst.tile([2 * H, VOC], f32, tag="ps")
    nc.tensor.matmul(ps1[:], w1s[:], embT[:], start=True, stop=True)
    r1 = cst.tile([2 * H, VOC], f32)
    nc.scalar.activation(r1[:], ps1[:], AF.Relu, bias=b1s[:], scale=1.0)
    ps2 = pst.tile([H, VOC], f32, tag="ps")
    nc.tensor.matmul(ps2[:], w2s[:], r1[:], start=True, stop=True)
    hpreT = cst.tile([H, VOC], f32)
    nc.vector.scalar_tensor_tensor(hpreT[:], ps2[:], b2s[:], embT[:],
                                   op0=AL.add, op1=AL.add)
    # transpose hpreT [H, VOC] -> hp [VOC, H] via DRAM roundtrip
    nc.sync.dma_start(tr_d[0:H, 0:VOC], hpreT[:])
    hp = cst.tile([VOC, H], f32)
    nc.sync.dma_start(hp[:], tr_d[0:H, 0:VOC].rearrange("a b -> b a"))
    mu = cst.tile([VOC, 1], f32)
    nc.vector.reduce_sum(mu[:], hp[:], axis=mybir.AxisListType.X)
    nc.vector.tensor_scalar(mu[:], mu[:], 1.0 / H, None, op0=AL.mult)
    xc = cst.tile([VOC, H], f32)
    nc.vector.tensor_scalar(xc[:], hp[:], mu[:], None, op0=AL.subtract)
    var = cst.tile([VOC, 1], f32)
    sq = cst.tile([VOC, H], f32)
    nc.vector.scalar_tensor_tensor(sq[:], xc[:], 1.0, xc[:],
                                   op0=AL.bypass, op1=AL.mult, accum_out=var[:])
    rstd = cst.tile([VOC, 1], f32)
    nc.vector.tensor_scalar(rstd[:], var[:], 1.0 / H, LN_EPS,
                            op0=AL.mult, op1=AL.add)
    nc.scalar.activation(rstd[:], rstd[:], AF.Sqrt)
    nc.vector.reciprocal(rstd[:], rstd[:])
    hn = cst.tile([VOC, H], f32)
    nc.vector.tensor_scalar(hn[:], xc[:], rstd[:], None, op0=AL.mult)
    nc.vector.tensor_mul(hn[:], hn[:], gRow[:])
    nc.vector.tensor_add(hn[:], hn[:], bRow[:])
    # transpose hn [VOC, H] -> hnT [H, VOC] via DRAM roundtrip
    nc.sync.dma_start(tr_d[0:VOC, 0:H], hn[:])
    hnT = cst.tile([H, VOC], f32)
    nc.sync.dma_start(hnT[:], tr_d[0:VOC, 0:H].rearrange("a b -> b a"))

    psk = pst.tile([VOC, 3 * H], f32, tag="ps")
    nc.tensor.matmul(psk[:, 0:H], hnT[:], wks[:], start=True, stop=True)
    nc.tensor.matmul(psk[:, H:2 * H], hnT[:], wvs[:], start=True, stop=True)
    nc.tensor.matmul(psk[:, 2 * H:3 * H], hnT[:], wqs[:], start=True, stop=True)
    kvq = cst.tile([VOC, 3 * H], f32)
    nc.vector.tensor_copy(kvq[:], psk[:])
    kn2 = cst.tile([VOC, 1], f32)
    ksq = cst.tile([VOC, H], f32)
    nc.vector.scalar_tensor_tensor(ksq[:], kvq[:, 0:H], 1.0, kvq[:, 0:H],
                                   op0=AL.bypass, op1=AL.mult, accum_out=kn2[:])
    rkn = cst.tile([VOC, 1], f32)
    nc.scalar.activation(rkn[:], kn2[:], AF.Sqrt)
    nc.vector.tensor_scalar(rkn[:], rkn[:], NORM_EPS, None, op0=AL.max)
    nc.vector.reciprocal(rkn[:], rkn[:])

    # kth: [Ktab_normalized | th]; vtab; qneg; ktabT
    kth = cst.tile([VOC, H + 1], f32)
    nc.vector.tensor_scalar(kth[:, 0:H], kvq[:, 0:H], rkn[:], None, op0=AL.mult)
    vtab = cst.tile([VOC, H], f32)
    nc.vector.tensor_copy(vtab[:], kvq[:, H:2 * H])
    th1 = cst.tile([VOC, 1], f32)
    vsq = cst.tile([VOC, H], f32)
    nc.vector.scalar_tensor_tensor(vsq[:], vtab[:], 1.0, vtab[:],
                                   op0=AL.bypass, op1=AL.mult, accum_out=th1[:])
    nc.vector.tensor_scalar(kth[:, H:H + 1], th1[:], THR2, None, op0=AL.mult)
    qneg = cst.tile([VOC, H], f32)
    nc.vector.tensor_scalar(qneg[:], kvq[:, 2 * H:3 * H], -1.0, None,
                            op0=AL.mult)
    # transpose kth[:, 0:H] [VOC, H] -> ktabT [H, VOC]
    nc.sync.dma_start(tr_d[0:VOC, 0:H], kth[:, 0:H])
    ktabT = cst.tile([H, VOC], f32)
    nc.sync.dma_start(ktabT[:], tr_d[0:VOC, 0:H].rearrange("a b -> b a"))

    # ============ persistent per-chunk tiles ============
    Mneg = per.tile([H, E, H], f32, tag="Mneg")    # -M^T per example
    nc.vector.memzero(Mneg[:])
    rbuf = per.tile([E, C, H], f32, tag="rbuf")    # r, then d in-place
    ubufT = per.tile([E, H, C], f32, tag="ubufT")  # gated u, [e, h, s]
    aneg2 = per.tile([E, C, C], f32, tag="aneg2")  # [e, t, s] = k_s . k_t
    tmpC = per.tile([E, H, C], f32, tag="tmpC")
    TK = per.tile([VOC, E, 2 * H], f32, tag="TK")
    thb = per.tile([E, C], f32, tag="thb")
    ncol = per.tile([E, 1], f32, tag="ncol")
    red0 = per.tile([E, H], f32, tag="red0")
    utt2 = per.tile([C, E, H], f32, tag="utt2")
    # scratch: oh [VOC, E, C] f32 | xb u8
    scratch = per.tile([VOC, 5 * E * C // 4], f32, tag="scratch")
    oh = scratch[:, 0:E * C].rearrange("v (e t) -> v e t", e=E, t=C)
    xb = scratch[:, E * C:5 * E * C // 4].bitcast(u8).rearrange(
        "v (e t) -> v e t", e=E, t=C)
    # reg: kt_sb [H+1, E, C] | a_sb [C, E, C] | rkstg [C, E, 2H]
    reg = per.tile([128, E * C + E * C + E * 2 * H], f32, tag="reg")
    kt_sb = reg[0:H + 1, 0:E * C].rearrange("p (e t) -> p e t", e=E, t=C)
    a_sb = reg[0:C, E * C:2 * E * C].rearrange("s (e t) -> s e t", e=E, t=C)
    rkstg = reg[0:C, 2 * E * C:2 * E * C + E * 2 * H].rearrange(
        "t (e h) -> t e h", e=E, h=2 * H)

    # TK[:, :, H:2H] = Ktab, shared across examples and chunks
    nc.scalar.copy(TK[:, :, H:2 * H],
                   kth[:, 0:H].unsqueeze(1).broadcast_to([VOC, E, H]))

    xf_r = xf.rearrange("e (n c) -> e n c", n=NCH, c=C)

    with tc.For_i(0, NCH) as ci:
        # ---- one-hot [VOC, (e, t)] ----
        nc.sync.dma_start(xb, xf_r[:, ci, :].unsqueeze(0)
                          .broadcast_to([VOC, E, C]))
        nc.vector.tensor_scalar(oh, xb, iotf[0:VOC, :], None, op0=AL.is_equal)

        # ---- K + th gather: 8 shared-stationary matmuls ----
        psKT = pst.tile([H + 1, E, C], f32, tag="ps")
        GK = max(1, 512 // C)             # examples per matmul (free dim <= 512)
        for b in range(E // GK):
            nc.tensor.matmul(psKT[:, GK * b:GK * (b + 1), :], kth[:],
                             oh[:, GK * b:GK * (b + 1), :], start=True, stop=True)
        nc.scalar.copy(kt_sb[:], psKT[:])
        nc.sync.dma_start(th_d, kt_sb[H:H + 1, :, :])
        nc.sync.dma_start(thb[:], th_d)

        # ---- A = K^T K per example -> aneg2 [e, t, s] (row-contiguous) ----
        psA = pst.tile([C, E, C], f32, tag="ps")
        for e in range(E):
            nc.tensor.matmul(psA[:, e, :], kt_sb[0:H, e, :], kt_sb[0:H, e, :],
                             start=True, stop=True)
        nc.scalar.copy(a_sb, psA[:])
        nc.sync.dma_start(aneg_d, a_sb)
        nc.sync.dma_start(aneg2[:], aneg_d.transpose([1, 2, 0]))

        # ---- T = Vtab - Ktab M^T ----
        psT = pst.tile([VOC, E, H], f32, tag="ps")
        for b in range(4):
            nc.tensor.matmul(psT[:, 8 * b:8 * b + 8, :], ktabT[:],
                             Mneg[:, 8 * b:8 * b + 8, :],
                             start=True, stop=True)
        nc.vector.tensor_tensor(TK[:, :, 0:H], psT[:],
                                vtab[:].unsqueeze(1)
                                .broadcast_to([VOC, E, H]), op=AL.add)

        # ---- r + K gather -> rbuf [e, t, h] via DRAM transpose ----
        psR = pst.tile([C, E, 2 * H], f32, tag="ps")
        for e in range(E):
            nc.tensor.matmul(psR[:, e, :], oh[:, e, :], TK[:, e, :],
                             start=True, stop=True)
        nc.scalar.copy(rkstg, psR[:])
        nc.sync.dma_start(rk_d, rkstg)
        nc.sync.dma_start(rbuf[:], rk_d[:, :, 0:H].transpose([1, 0, 2]))

        # ---- solve: sequential gated forward substitution, W=1 ----
        nc.vector.memzero(ubufT[:])
        with tc.For_i(0, C) as t:
            # correction: d_t = r_t - sum_s A[t,s] * u_s  (u_s = 0 for s >= t)
            nc.vector.tensor_tensor(
                tmpC[:], ubufT[:],
                aneg2[:, t, :].unsqueeze(1).broadcast_to([E, H, C]),
                op=AL.mult)
            nc.vector.reduce_sum(red0[:], tmpC[:], axis=mybir.AxisListType.X)
            nc.vector.tensor_tensor(rbuf[:, t, :], rbuf[:, t, :], red0[:],
                                    op=AL.subtract)
            # gate: ||d||^2 > th  ->  u_t = g * d
            nc.vector.scalar_tensor_tensor(red0[:], rbuf[:, t, :], 1.0,
                                           rbuf[:, t, :], op0=AL.bypass,
                                           op1=AL.mult, accum_out=ncol[:])
            nc.vector.scalar_tensor_tensor(
                ubufT[:, :, t], ncol[:].broadcast_to([E, H]),
                thb[:, t].unsqueeze(1), rbuf[:, t, :],
                op0=AL.is_gt, op1=AL.mult)

        # ---- M update: Mneg -= K^T U ----
        nc.sync.dma_start(utt_d, ubufT[:])
        nc.sync.dma_start(utt2[:], utt_d.transpose([2, 0, 1]))
        psM = pst.tile([H, E, H], f32, tag="ps")
        for e in range(E):
            nc.tensor.matmul(psM[:, e, :], rkstg[:, e, H:2 * H],
                             utt2[:, e, :], start=True, stop=True)
        nc.vector.tensor_tensor(Mneg[:].rearrange("h e j -> h (e j)"),
                                Mneg[:].rearrange("h e j -> h (e j)"),
                                psM[:].rearrange("h e j -> h (e j)"),
                                op=AL.subtract)

    # ============ readout ============
    xlb = per.tile([VOC, E], u8, tag="xlb")
    nc.sync.dma_start(xlb[:], xf[:, L - 1].unsqueeze(0).broadcast_to([VOC, E]))
    ohl = per.tile([VOC, E], f32, tag="ohl")
    nc.vector.tensor_scalar(ohl[:], xlb[:], iotf[0:VOC, :], None,
                            op0=AL.is_equal)
    psq = pst.tile([H, E], f32, tag="ps")
    nc.tensor.matmul(psq[:], qneg[:], ohl[:], start=True, stop=True)
    qng = per.tile([H, E], f32, tag="qng")
    nc.scalar.copy(qng[:], psq[:])
    prd = pst.tile([H, E], f32, tag="ps")
    for e in range(E):
        nc.tensor.matmul(prd[:, e:e + 1], Mneg[:, e, :],
                         qng[:, e:e + 1], start=True, stop=True)
    rd = qng
    nc.scalar.activation(rd[:], prd[:], AF.Relu)
    pso = pst.tile([H, E], f32, tag="ps")
    nc.tensor.matmul(pso[:], wos[:], rd[:], start=True, stop=True)
    ot = per.tile([H, E], f32, tag="ot")
    nc.vector.tensor_scalar(ot[:], pso[:], bos[:], None, op0=AL.add)
    nc.sync.dma_start(out_d, ot[:])


def make_consts(inputs):
    consts = {
        "embed": inputs["embed"], "w1": inputs["w1"],
        "b1": np.asarray(inputs["b1"]).reshape(2 * H, 1),
        "w2": inputs["w2"], "b2": np.asarray(inputs["b2"]).reshape(H, 1),
        "ln_g": np.asarray(inputs["ln_g"]).reshape(1, H),
        "ln_b": np.asarray(inputs["ln_b"]).reshape(1, H),
        "wk": inputs["wk"], "wv": inputs["wv"], "wq": inputs["wq"],
        "wo": inputs["wo"], "bo": np.asarray(inputs["bo"]).reshape(H, 1),
    }
    consts = {k: np.ascontiguousarray(np.asarray(v, dtype=np.float32))
              for k, v in consts.items()}
    consts["iotaf"] = (np.arange(128) % 64).astype(np.float32).reshape(128, 1)
    blob = np.concatenate(
        [consts[n].reshape(-1) for n, _ in CONST_SPECS]).astype(np.float32)
    assert blob.size == NBLOB
    return blob.reshape(1, NBLOB)


def build_nc(inputs=None):
    from concourse import bacc
    import concourse.tile as tile
    from contextlib import ExitStack
    nc = bacc.Bacc("TRN2", target_bir_lowering=False, debug=False,
                   num_devices=NCORE)
    with tile.TileContext(nc) as tc:
        with ExitStack() as ctx:
            _build(nc, tc, ctx)
    nc.compile()
    return nc


def make_in_maps(inputs):
    x = np.asarray(inputs["x"]).astype(np.int64)
    blob = make_consts(inputs)
    in_maps = []
    for c in range(NCORE):
        w = NBLOB // NCORE
        m = {"cpart": np.ascontiguousarray(blob[:, c * w:(c + 1) * w]),
             "xf": np.ascontiguousarray(x[c * BPC:(c + 1) * BPC]
                                        .astype(np.uint8))}
        in_maps.append(m)
    return in_maps


def kernel(**inputs):
    from concourse.bass_utils import run_bass_kernel_spmd
    nc = build_nc()
    in_maps = make_in_maps(inputs)
    res = run_bass_kernel_spmd(nc, in_maps, list(range(NCORE)))
    outs = []
    for c in range(NCORE):
        o = np.asarray(res.results[c]["out"])   # [H, BPC]
        outs.append(o.T)
    return np.concatenate(outs, axis=0).astype(np.float32)
